# revision 1
# baseline (speedup 1.0000x reference)
"""CorrelationLayer Trainium2 kernel (8-core SPMD, data-parallel over batch).

Reference computation:
    f1n = l2_normalize(feat1, axis=C); f2n = l2_normalize(feat2, axis=C)
    corr[b, q=(dy,dx), h, w] = sum_c f1n[b,c,h,w] * f2n_padded[b,c,h+dy,w+dx]
    for dy,dx in [-4,4]^2 (81 displacements), zero padding of 4.

Strategy per core (2 batch images):
  - Band matmul on TensorE (fp32r): per (image, h-tile 16 x w-tile 8) the
    stationary operand is the 128-pixel f1 tile (contraction over C=2x128),
    the moving operand is the 24x16 padded window of normalized f2
    (N=384 columns).  Output band (128, 384) in PSUM holds every pixel's 81
    correlations at band position n_q(p) = psi(p) + 16*(dy+4) + (dx+4),
    psi(p) = 16*lh + lw  (a per-partition "diagonal" layout).
  - Diagonal extraction is impossible with partition-uniform access patterns,
    so the band is written to a DRAM scratch with an affine *sheared* AP
    (row p shifted left by psi(p); contiguous 1536 B runs) and read back as a
    (128, 137) window in which displacement q sits at the partition-uniform
    offset 16*(dy+4)+(dx+4).  A strided-AP DVE copy compacts it to (128, 81).
  - f2 is l2-normalized on the fly in SBUF: sum-of-squares via ACT square +
    ones-matmul (broadcast rows in PSUM), sqrt+reciprocal on a compact
    (128, n) layout via a tiny DRAM bounce, re-broadcast via a K=1
    ones-matmul, multiply on DVE.  f1's inverse norm is applied as the
    per-partition ACT scale during the PSUM->SBUF band copy.
  - Output is written as (b, ht, wt, 128, 81) tiles; the host reassembles
    the (B, 81, H, W) layout.
"""

import numpy as np

import concourse.bass as bass
import concourse.mybir as mybir
import concourse.tile as tile
from concourse.vector_clock import ScopedClock

# ---------------------------------------------------------------------------
# Problem constants (hardcoded per spec).
B, C, H, W = 16, 256, 96, 128
NCORES = 8
BL = B // NCORES          # batch per core
CB = 2                    # C blocks of 128
TH, TW = 16, 8            # f1 tile (128 pixels)
SH, SW = TH + 8, TW + 8   # f2 stream window 24 x 16
N = SH * SW               # 384 moving columns
HT, WT = H // TH, W // TW # 6 x 16 tiles per image
JROW = 631                # sheared scratch row length (>= N + psi_max = 384+247)
PSI_MAX = 16 * (TH - 1) + (TW - 1)  # 247
WIN = N - PSI_MAX         # 137 readback window
Q = 81
F32 = mybir.dt.float32
F32R = mybir.dt.float32r

NORM_EPS_SQ = 1e-30       # clamp for sqrt(ss) on zero-padded pixels


# ---------------------------------------------------------------------------
# Workarounds for this walrus build: at most ONE sync-wait per instruction.
def _drain_and_barrier(self, tick_clock, wait_clock):
    nc = self.nc
    drain_inst = nc.sync.drain()
    wait_clock.add_sem_waits(
        drain_inst.ins, ScopedClock({None: tick_clock.global_clock})
    )
    si = drain_inst.ins.sync_info
    if si is not None and si.on_wait and len(si.on_wait) > 1:
        waits = list(si.on_wait)
        drain_inst.ins.sync_info = mybir.SyncInfo(
            on_wait=[waits[0]], on_update=list(si.on_update or [])
        )
        for w in waits[1:]:
            n = nc.sync.nop(nofuse=True)
            n.ins.sync_info = mybir.SyncInfo(on_wait=[w], on_update=[])
    nc.all_engine_barrier()
    assert self.sems is not None
    popped = nc._tile_sem_poison_stack.pop()
    assert popped is self._sem_poison
    nc.clear_and_free_semaphores(list(self.sems.allocated().values()))
    nc.all_engine_barrier()


tile.TileContext._drain_and_barrier = _drain_and_barrier


def split_multi_waits(nc):
    """Move extra sync-waits onto same-engine nops inserted just before."""
    counter = 0
    for fn in nc.m.functions:
        for bb in fn.blocks:
            new = []
            for inst in bb.instructions:
                si = inst.sync_info
                if si is not None and si.on_wait and len(si.on_wait) > 1:
                    waits = list(si.on_wait)
                    for w in waits[:-1]:
                        counter += 1
                        nop = mybir.InstNoOp(
                            name=f"I-waitsplit-{counter}", ins=[], outs=[]
                        )
                        nop.engine = inst.engine
                        nop.sync_info = mybir.SyncInfo(on_wait=[w], on_update=[])
                        new.append(nop)
                    inst.sync_info = mybir.SyncInfo(
                        on_wait=[waits[-1]], on_update=list(si.on_update or [])
                    )
                new.append(inst)
            bb.instructions = new


# ---------------------------------------------------------------------------
def build_program(repeats: int = 1):
    """Build the per-core Bass program.  `repeats` re-runs the whole kernel
    body (for wall-clock timing in the dev harness)."""
    nc = bass.Bass("TRN2", target_bir_lowering=False, debug=False)
    f1d = nc.declare_dram_parameter("feat1", [BL, C, H, W], F32, isOutput=False)
    f2d = nc.declare_dram_parameter("feat2", [BL, C, H, W], F32, isOutput=False)
    outd = nc.declare_dram_parameter("out", [BL, HT, WT, 128, Q], F32, isOutput=True)

    with tile.TileContext(nc) as tc:
        with (
            tc.tile_pool(name="const", bufs=1) as cpool,
            tc.tile_pool(name="bandio", bufs=2) as bpool,
            tc.tile_pool(name="sq", bufs=2) as sqpool,
            tc.tile_pool(name="norm", bufs=1) as npool,
            tc.tile_pool(name="tiles", bufs=3) as tpool,
            tc.tile_pool(name="stgp", bufs=2) as stgpool,
            tc.tile_pool(name="psmm", bufs=3, space="PSUM") as psmm,
            tc.tile_pool(name="psnorm", bufs=2, space="PSUM") as psnorm,
            tc.tile_pool(name="dscr", bufs=4, space="DRAM") as dpool,
            tc.tile_pool(name="dnorm", bufs=2, space="DRAM") as dnpool,
        ):
            epsb = cpool.tile([128, 1], F32)
            nc.vector.memset(epsb[:], NORM_EPS_SQ)
            ones = cpool.tile([128, 128], F32R)
            ones_dram = nc.inline_tensor(np.ones((128, 128), np.float32), "ones_c")
            nc.sync.dma_start(ones[:], ones_dram.ap().bitcast(F32R))
            onesr = ones[:]

            NPIX2 = SH * (W + 8)          # f2 band pixels incl pads: 24*136=3264
            NPIX2_PAD = 3584              # rounded up to 7*512
            NPIX1 = TH * W                # f1 band pixels: 2048

            for _ in range(repeats):
                for b in range(BL):
                    for ht in range(HT):
                        h0 = ht * TH
                        # ---------------- loads ----------------
                        f1b = bpool.tile([128, CB, TH, W], F32R, tag="f1b")
                        for cb in range(CB):
                            nc.sync.dma_start(
                                f1b[:, cb],
                                f1d[b, cb * 128:(cb + 1) * 128,
                                    h0:h0 + TH, :].bitcast(F32R),
                            )
                        f2b = bpool.tile([128, CB, SH, W + 8], F32R, tag="f2b")
                        r0 = max(0, h0 - 4)
                        r1 = min(H, h0 + TH + 4)
                        lo = r0 - (h0 - 4)          # first valid band row
                        hi = lo + (r1 - r0)
                        f2b32 = f2b[:].bitcast(F32)
                        # zero pads: w borders (all rows) + h edge rows
                        nc.gpsimd.memset(f2b32[:, :, :, 0:4], 0.0)
                        nc.gpsimd.memset(f2b32[:, :, :, W + 4:W + 8], 0.0)
                        if lo > 0:
                            nc.gpsimd.memset(f2b32[:, :, 0:lo, :], 0.0)
                        if hi < SH:
                            nc.gpsimd.memset(f2b32[:, :, hi:SH, :], 0.0)
                        for cb in range(CB):
                            nc.sync.dma_start(
                                f2b32[:, cb, lo:hi, 4:W + 4],
                                f2d[b, cb * 128:(cb + 1) * 128, r0:r1, :],
                            )

                        # ---------------- f2 norm ----------------
                        # ss2 broadcast rows per 512-pixel chunk
                        f2flat = f2b32.rearrange("c cb h w -> c cb (h w)")
                        sqs2 = npool.tile([128, NPIX2_PAD], F32, tag="sqs2")
                        nchunk2 = (NPIX2 + 511) // 512  # 7
                        for ch in range(nchunk2):
                            p0 = ch * 512
                            p1 = min(NPIX2, p0 + 512)
                            cw = p1 - p0
                            ssps = psnorm.tile([128, 512], F32, tag="ssps")
                            for cb in range(CB):
                                sq = sqpool.tile([128, 512], F32R, tag="sq")
                                if ch % 2 == 0:
                                    nc.scalar.activation(
                                        sq[:, :cw],
                                        f2flat[:, cb, p0:p1],
                                        mybir.ActivationFunctionType.Square,
                                    )
                                else:
                                    nc.vector.tensor_mul(
                                        sq[:, :cw],
                                        f2flat[:, cb, p0:p1],
                                        f2flat[:, cb, p0:p1],
                                    )
                                nc.tensor.matmul(
                                    ssps[:, :cw], onesr, sq[:, :cw],
                                    start=(cb == 0), stop=(cb == CB - 1),
                                )
                            # evacuate (plain copy, alternate engines)
                            if ch % 2 == 0:
                                nc.vector.tensor_copy(
                                    sqs2[:, p0:p1], ssps[:, :cw]
                                )
                            else:
                                nc.scalar.copy(sqs2[:, p0:p1], ssps[:, :cw])
                        if NPIX2_PAD > NPIX2:
                            nc.vector.memset(sqs2[:, NPIX2:], 1.0)

                        # bounce row0 -> DRAM, read compact, rsqrt, bounce back
                        n2row_d = dnpool.tile([NPIX2_PAD], F32, tag="n2d")
                        nc.sync.dma_start(
                            n2row_d[:].rearrange("(a x) -> a x", a=1),
                            sqs2[0:1, :],
                        )
                        n2c = npool.tile([128, NPIX2_PAD // 128], F32, tag="n2c")
                        nc.sync.dma_start(
                            n2c[:],
                            bass.AP(n2row_d.tensor, n2row_d.offset,
                                    [[NPIX2_PAD // 128, 128],
                                     [1, NPIX2_PAD // 128]]),
                        )
                        # inv2 = 1/sqrt(ss + eps)
                        nc.scalar.activation(
                            n2c[:], n2c[:], mybir.ActivationFunctionType.Sqrt,
                            bias=epsb[:],
                        )
                        nc.vector.reciprocal(n2c[:], n2c[:])
                        inv2row_d = dnpool.tile([NPIX2_PAD], F32, tag="i2d")
                        nc.sync.dma_start(
                            bass.AP(inv2row_d.tensor, inv2row_d.offset,
                                    [[NPIX2_PAD // 128, 128],
                                     [1, NPIX2_PAD // 128]]),
                            n2c[:],
                        )
                        inv2row = npool.tile([1, NPIX2_PAD], F32R, tag="i2r")
                        nc.sync.dma_start(
                            inv2row[:],
                            inv2row_d[:].rearrange("(a x) -> a x", a=1)
                            .bitcast(F32R),
                        )
                        # rebroadcast + multiply f2 in place (f2n, fp32r out)
                        for ch in range(nchunk2):
                            p0 = ch * 512
                            p1 = min(NPIX2, p0 + 512)
                            cw = p1 - p0
                            bcps = psnorm.tile([128, 512], F32, tag="bcps")
                            nc.tensor.matmul(
                                bcps[:, :cw], onesr[0:1, :],
                                inv2row[:, p0:p1],
                                start=True, stop=True,
                            )
                            f2r = f2b[:].rearrange("c cb h w -> c cb (h w)")
                            for cb in range(CB):
                                nc.vector.tensor_mul(
                                    f2r[:, cb, p0:p1],
                                    f2flat[:, cb, p0:p1],
                                    bcps[:, :cw],
                                )

                        # ---------------- f1 norm (inv1) ----------------
                        f1flat = f1b[:].bitcast(F32).rearrange(
                            "c cb h w -> c cb (h w)"
                        )
                        sqs1 = npool.tile([128, NPIX1], F32, tag="sqs1")
                        for ch in range(NPIX1 // 512):  # 4
                            p0 = ch * 512
                            ssps = psnorm.tile([128, 512], F32, tag="ssps")
                            for cb in range(CB):
                                sq = sqpool.tile([128, 512], F32R, tag="sq")
                                if ch % 2 == 1:
                                    nc.scalar.activation(
                                        sq[:],
                                        f1flat[:, cb, p0:p0 + 512],
                                        mybir.ActivationFunctionType.Square,
                                    )
                                else:
                                    nc.vector.tensor_mul(
                                        sq[:],
                                        f1flat[:, cb, p0:p0 + 512],
                                        f1flat[:, cb, p0:p0 + 512],
                                    )
                                nc.tensor.matmul(
                                    ssps[:], onesr, sq[:],
                                    start=(cb == 0), stop=(cb == CB - 1),
                                )
                            if ch % 2 == 0:
                                nc.scalar.copy(sqs1[:, p0:p0 + 512], ssps[:])
                            else:
                                nc.vector.tensor_copy(
                                    sqs1[:, p0:p0 + 512], ssps[:]
                                )
                        n1row_d = dnpool.tile([NPIX1], F32, tag="n1d")
                        nc.sync.dma_start(
                            n1row_d[:].rearrange("(a x) -> a x", a=1),
                            sqs1[0:1, :],
                        )
                        n1c = npool.tile([128, NPIX1 // 128], F32, tag="n1c")
                        nc.sync.dma_start(
                            n1c[:],
                            bass.AP(n1row_d.tensor, n1row_d.offset,
                                    [[NPIX1 // 128, 128], [1, NPIX1 // 128]]),
                        )
                        nc.scalar.activation(
                            n1c[:], n1c[:], mybir.ActivationFunctionType.Sqrt,
                            bias=epsb[:],
                        )
                        nc.vector.reciprocal(n1c[:], n1c[:])
                        inv1row_d = dnpool.tile([NPIX1], F32, tag="i1d")
                        nc.sync.dma_start(
                            bass.AP(inv1row_d.tensor, inv1row_d.offset,
                                    [[NPIX1 // 128, 128], [1, NPIX1 // 128]]),
                            n1c[:],
                        )

                        # ------------- f1 re-tile (contiguous lhsT) -------------
                        f1s = bpool.tile([128, CB, WT, TH * TW], F32R, tag="f1s")
                        for cb in range(CB):
                            src_ap = bass.AP(
                                f1b[:].tensor,
                                f1b[:].offset + cb * (TH * W),
                                [[CB * TH * W, 128], [TW, WT], [W, TH], [1, TW]],
                            )
                            nc.vector.tensor_copy(f1s[:, cb], src_ap)

                        # ---------------- tiles ----------------
                        for wt in range(WT):
                            w0 = wt * TW
                            # stage rhs window contiguously (fp32r out)
                            stg = stgpool.tile([128, CB, SH * SW], F32R, tag="stg")
                            for cb in range(CB):
                                win = f2b[:, cb, :, w0:w0 + SW]
                                if (wt + cb) % 2 == 0:
                                    nc.vector.tensor_copy(stg[:, cb], win)
                                else:
                                    nc.scalar.copy(stg[:, cb], win)
                            ps = psmm.tile([128, N], F32, tag="band")
                            for cb in range(CB):
                                nc.tensor.matmul(
                                    ps[:], f1s[:, cb, wt], stg[:, cb],
                                    start=(cb == 0), stop=(cb == CB - 1),
                                )
                            # inv1 tile (128,1): raster idx = lh*128 + w0 + lw
                            inv1t = tpool.tile([128, 1], F32, tag="inv1t")
                            nc.sync.dma_start(
                                inv1t[:],
                                bass.AP(inv1row_d.tensor,
                                        inv1row_d.offset + w0,
                                        [[W, TH], [1, TW]]),
                            )
                            band = tpool.tile([128, N], F32, tag="band_sb")
                            nc.scalar.activation(
                                band[:], ps[:],
                                mybir.ActivationFunctionType.Copy,
                                scale=inv1t[:],
                            )
                            # shear write
                            dsc = dpool.tile([128 * JROW], F32, tag="dsc")
                            dst = bass.AP(
                                dsc.tensor, dsc.offset + PSI_MAX,
                                [[TW * JROW - SW, TH], [JROW - 1, TW], [1, N]],
                            )
                            nc.sync.dma_start(dst, band[:])
                            # readback window
                            d3 = tpool.tile([128, WIN], F32, tag="d3")
                            nc.sync.dma_start(
                                d3[:],
                                bass.AP(dsc.tensor, dsc.offset + PSI_MAX,
                                        [[JROW, 128], [1, WIN]]),
                            )
                            # compact extract (q = 9*(dy') + dx')
                            ot = tpool.tile([128, 9, 9], F32, tag="ot")
                            nc.vector.tensor_copy(
                                ot[:],
                                bass.AP(d3[:].tensor, d3[:].offset,
                                        [[WIN, 128], [SW, 9], [1, 9]]),
                            )
                            nc.sync.dma_start(
                                outd[b, ht, wt],
                                ot[:].rearrange("p a b -> p (a b)"),
                            )

    split_multi_waits(nc)
    return nc


# ---------------------------------------------------------------------------
_CACHE = {}


def _get_runner():
    if "runner" not in _CACHE:
        _CACHE["runner"] = build_program(repeats=1)
    return _CACHE["runner"]


def kernel(feat1, feat2):
    from concourse.bass_utils import run_bass_kernel_spmd

    feat1 = np.asarray(feat1, dtype=np.float32)
    feat2 = np.asarray(feat2, dtype=np.float32)
    assert feat1.shape == (B, C, H, W) and feat2.shape == (B, C, H, W)

    nc = _get_runner()
    in_maps = [
        {
            "feat1": feat1[core * BL:(core + 1) * BL],
            "feat2": feat2[core * BL:(core + 1) * BL],
        }
        for core in range(NCORES)
    ]
    res = run_bass_kernel_spmd(nc, in_maps, list(range(NCORES)))

    # reassemble (BL, HT, WT, 128, 81) tiles -> (B, 81, H, W)
    out = np.empty((B, Q, H, W), dtype=np.float32)
    for core in range(NCORES):
        t = res.results[core]["out"]  # (BL, HT, WT, 128, 81)
        t = t.reshape(BL, HT, WT, TH, TW, Q)
        # -> (BL, Q, HT, TH, WT, TW)
        t = t.transpose(0, 5, 1, 3, 2, 4).reshape(BL, Q, H, W)
        out[core * BL:(core + 1) * BL] = t
    return out

